# revision 1
# baseline (speedup 1.0000x reference)
"""Multi-head causal attention (d_model=768, 12 heads, seq 2048, batch 2) on
8 Trainium2 NeuronCores.

Sharding: tensor-parallel over heads x data-parallel over batch.
Core c handles batch b = c // 4 and heads [3*(c%4), 3*(c%4)+3).
Each core computes its 3 heads' attention plus its partial output
projection; the host sums the 4 partials per batch and adds the bias
(f32 reduction on host, partials written bf16).

Design (all-bf16 datapath; PE ref: warm 2.4 GHz, ~N cycles per matmul
of N moving columns):
  - Everything PE-facing is bf16 (same PE speed as f32r at N>=192 by
    measurement, half the DMA bytes, ~3e-3 rel err by simulation).
  - Scores matmuls have only K=64 real contraction, so two are packed
    into the 128-row array concurrently via tile_position row groups
    (measured 2x: 108.8 ns per member at N=512). Heads 0,1 pair
    naturally (Q/K stacked in one 128-partition tile); head 2 pairs
    its own chunks j/j+1 against partition-duplicated Q2/K2 copies.
  - V is computed in natural [t, dk] layout directly (stationary = xT
    chunk, moving = Wv, N=192 full rate) - no PE transposes.
  - Scores land in 2-bank PSUM tiles ([128, 2, 512]); exp runs as ONE
    ACTIVATE per pair (amortizes the 352-cycle ACT overhead; the exp
    stream is the ACT-side floor of the whole kernel).
  - Causal masking: skipped t-blocks are never computed; diagonal
    triangles (and head-2 member-B stale columns) are zeroed AFTER exp
    by GPSIMD affine_select (select, not multiply - exp(junk) safe).
  - Row sums l come from a ones-column appended to V ([V|1]).
    Normalization: O/l stage to SBUF in two quick DVE copies (frees
    the PSUM bank), reciprocal_approx_fast, GPSIMD partition_broadcast;
    the broadcast-dependent multiply is deferred past the next QKV
    block's evacuations so it never blocks the DVE FIFO head.
  - Emission order per super-block: h01 pair stream, h2 pair stream
    (before the next QKV block - avoids a coarse-grained false
    dependency of h2's score reads on the next q2d/k2d writes), then
    QKV for the next t-block, then the deferred norm multiplies.
  - DMA: inputs host-prepped chunk-blocked (fully contiguous per
    transfer); doorbells alternate sync/scalar HWDGE queues; output
    rows split across both queues. QKV for t-block 0 accumulates
    k-outer so matmuls start as soon as chunk 0 lands.
  - Output projection (ot01 K=128 + zero-padded ot2 K=128 against
    host-presliced Wo) is dripped through the ACT-bound attention
    stretches; the last 4 blocks alternate PSUM homes and evacuate
    via DVE+ACT for a 2-way overlapped tail.
"""

import sys
import types

import numpy as np
import ml_dtypes

import concourse.bass as bass
import concourse.tile as tile
from concourse import mybir, bacc
from concourse.bass_utils import run_bass_kernel_spmd

# Register the axon NTFF profiling hook if the environment supports it, so
# running with BASS_TRACE=1 yields exec_time_ns instead of an import error.
try:
    import antenv.axon_hooks  # noqa: F401
except ImportError:
    try:
        from trn_agent_boot.trn_boot import _ntff_profile_via_ctypes

        _hook = _ntff_profile_via_ctypes("/opt/axon/libaxon_pjrt.so")
        _mod = types.ModuleType("antenv.axon_hooks")
        _mod.get_axon_ntff_profile_hook = lambda: _hook
        _mod.set_axon_ntff_profile_hook = lambda h: None
        sys.modules["antenv.axon_hooks"] = _mod
    except Exception:
        pass

F32 = mybir.dt.float32
BF16 = mybir.dt.bfloat16

N_CORES = 8
B = 2
S = 2048
D = 768
H = 12
DK = 64
H_PER_CORE = 3
NSUP = S // 512  # 4 q super-blocks of 512
NKCH = D // 128  # 6 contraction chunks
SCALE = 0.125  # 1/sqrt(64)

_CACHED_NC = None


def build_bass():
    nc = bacc.Bacc()
    xT = nc.declare_dram_parameter("xT", [NKCH, 128, S], BF16, isOutput=False)
    # wqk chunk columns: [Wq0|Wq1], [Wk0|Wk1], [Wq2|Wk2]
    wqk = nc.declare_dram_parameter("wqk", [NKCH, 128, 384], BF16, isOutput=False)
    wv = nc.declare_dram_parameter("wv", [NKCH, 128, 192], BF16, isOutput=False)
    w2 = nc.declare_dram_parameter("w2", [192, D], BF16, isOutput=False)
    out = nc.declare_dram_parameter("out", [S, D], BF16, isOutput=True)

    with tile.TileContext(nc) as tc:
        with (
            tc.tile_pool(name="persist", bufs=1) as pers,
            tc.tile_pool(name="ptpool", bufs=4) as ptpool,
            tc.tile_pool(name="norm", bufs=3) as norm,
            tc.tile_pool(name="stage", bufs=3) as stage,
            tc.tile_pool(name="ps_sc", bufs=2, space="PSUM") as ps_sc,
            tc.tile_pool(name="ps_ot", bufs=2, space="PSUM") as ps_ot,
            tc.tile_pool(name="ps_pp", bufs=1, space="PSUM") as ps_pp,
        ):
            # ---- persistent SBUF tiles ----
            xT_sb = [pers.tile([128, S], BF16, tag=f"xt{k}", name=f"xt{k}") for k in range(NKCH)]
            wqk_sb = [pers.tile([128, 384], BF16, tag=f"wqk{k}", name=f"wqk{k}") for k in range(NKCH)]
            wv_sb = [pers.tile([128, 192], BF16, tag=f"wv{k}", name=f"wv{k}") for k in range(NKCH)]
            w2a_sb = pers.tile([128, D], BF16, tag="w2a")
            w2b_sb = pers.tile([128, D], BF16, tag="w2b")
            q01 = pers.tile([128, S], BF16, tag="q01")  # [Q0.T ; Q1.T]
            k01 = pers.tile([128, S], BF16, tag="k01")  # [K0.T ; K1.T]
            q2d = pers.tile([128, S], BF16, tag="q2d")  # [Q2.T ; Q2.T]
            k2d = pers.tile([128, S], BF16, tag="k2d")  # [K2.T ; K2.T]
            # V natural + ones column: [t-part, block, head, 65]
            vn = pers.tile([128, S // 128, H_PER_CORE, 65], BF16, tag="vn")
            ot01 = pers.tile([128, S], BF16, tag="ot01")
            ot2 = pers.tile([128, S], BF16, tag="ot2")  # rows 64:128 zero

            # warm-up source first in the DVE queue (gates the first matmul)
            wsrc = pers.tile([128, 512], BF16, tag="wsrc")
            nc.vector.memset(wsrc, 1.0)
            # exp table preload: tiny dummy activation during the DMA wait
            dumm = pers.tile([1, 16], F32, tag="dumm")
            dumo = pers.tile([1, 16], BF16, tag="dumo")
            nc.vector.memset(dumm, 0.0)
            nc.scalar.activation(out=dumo, in_=dumm,
                                 func=mybir.ActivationFunctionType.Exp, scale=1.0)

            # ---- input DMAs: doorbells alternate sync / scalar queues ----
            def dq(i):
                return (nc.sync, nc.scalar)[i % 2]

            for k in range(NKCH):
                dq(k).dma_start(out=wqk_sb[k], in_=wqk[k, :, :])
                dq(k + 1).dma_start(out=wv_sb[k], in_=wv[k, :, :])
                dq(k).dma_start(out=xT_sb[k], in_=xT[k, :, :])
            nc.scalar.dma_start(out=w2a_sb, in_=w2[0:128, :])
            nc.vector.memset(w2b_sb, 0.0)
            nc.sync.dma_start(out=w2b_sb[0:64, :], in_=w2[128:192, :])

            nc.gpsimd.memset(ot2[64:128, :], 0.0)
            nc.vector.memset(vn[:, :, :, 64:65], 1.0)

            # PE warm-up: keeps the HAM activity monitor busy during the
            # input DMA wait (result discarded)
            wps = ps_pp.tile([128, 2, 512], F32, tag="pp", name="wps")
            for wi in range(10):
                nc.tensor.matmul(
                    wps[:, 0, :], wsrc[:, 0:128], wsrc,
                    start=(wi == 0), stop=(wi == 9), skip_group_check=True,
                )

            # ---- QKV evacuation helpers (all PSUM reads on DVE) ----
            def evac_qk(c, pt, ncols):
                if c == 0:
                    nc.vector.tensor_copy(q01[:, ncols], pt)
                elif c == 1:
                    nc.vector.tensor_copy(k01[:, ncols], pt)
                else:
                    nc.vector.tensor_copy(q2d[0:64, ncols], pt[0:64, :])
                    nc.vector.tensor_copy(q2d[64:128, ncols], pt[0:64, :])
                    nc.vector.tensor_copy(k2d[0:64, ncols], pt[64:128, :])
                    nc.vector.tensor_copy(k2d[64:128, ncols], pt[64:128, :])

            # ---- QKV for t-block 0: k-outer so matmuls start on chunk 0 ----
            def emit_qkv0():
                ncols = bass.ts(0, 512)
                sc0 = ps_sc.tile([128, 2, 512], F32, tag="sc", name="qsc0")
                sc1 = ps_sc.tile([128, 2, 512], F32, tag="sc", name="qsc1")
                po0 = ps_ot.tile([128, 512], F32, tag="ot", name="qpo0")
                po1 = ps_ot.tile([128, 512], F32, tag="ot", name="qpo1")
                pp = ps_pp.tile([128, 2, 512], F32, tag="pp", name="qpp")
                cts = [sc0[:, 0, :], sc0[:, 1, :], sc1[:, 0, :]]
                vts = [po0[:, 0:192], po1[:, 0:192], pp[:, 0, 0:192], pp[:, 1, 0:192]]
                for k in range(NKCH):
                    st, sp = k == 0, k == NKCH - 1
                    for c in range(3):
                        nc.tensor.matmul(
                            cts[c], wqk_sb[k][:, 128 * c : 128 * c + 128],
                            xT_sb[k][:, ncols], start=st, stop=sp,
                            skip_group_check=True,
                        )
                    for b_ in range(4):
                        nc.tensor.matmul(
                            vts[b_], xT_sb[k][:, bass.ts(b_, 128)], wv_sb[k],
                            start=st, stop=sp, skip_group_check=True,
                        )
                for c in range(3):
                    evac_qk(c, cts[c], ncols)
                for b_ in range(4):
                    nc.vector.tensor_copy(vn[:, b_, :, 0:64], vts[b_])

            # ---- QKV for t-blocks 1..3: c-outer on pp/ot pools ----
            def emit_qkv(nt):
                ncols = bass.ts(nt, 512)
                pp = ps_pp.tile([128, 2, 512], F32, tag="pp", name="qkpp")

                def cgroup(c):
                    pt = pp[:, c, :] if c < 2 else ps_ot.tile(
                        [128, 512], F32, tag="ot", name="qkot")
                    for k in range(NKCH):
                        nc.tensor.matmul(
                            pt, wqk_sb[k][:, 128 * c : 128 * c + 128],
                            xT_sb[k][:, ncols],
                            start=(k == 0), stop=(k == NKCH - 1),
                        )
                    evac_qk(c, pt, ncols)

                def vgroup2(blk):
                    # two t-blocks accumulate in one bank, one paired evac
                    po = ps_ot.tile([128, 512], F32, tag="ot", name="qv")
                    for i in range(2):
                        pv = po[:, 192 * i : 192 * i + 192]
                        for k in range(NKCH):
                            nc.tensor.matmul(
                                pv, xT_sb[k][:, bass.ts(blk + i, 128)], wv_sb[k],
                                start=(k == 0), stop=(k == NKCH - 1),
                            )
                    nc.vector.tensor_copy(vn[:, blk : blk + 2, :, 0:64],
                                          po[:, 0:384])

                b0 = 4 * nt
                cgroup(0)
                vgroup2(b0)
                cgroup(1)
                vgroup2(b0 + 2)
                cgroup(2)

            # ---- output projection, per 128-row q block ----
            def emit_one_op(qb, alt=False, act_copy=False):
                qs = bass.ts(qb, 128)
                if alt:
                    t1 = ps_ot.tile([128, 512], F32, tag="ot", name="opa")
                    t2 = ps_ot.tile([128, 512], F32, tag="ot", name="opb")
                    dsts = [t1[:, 0:512], t2[:, 0:256]]
                else:
                    pp = ps_pp.tile([128, 2, 512], F32, tag="pp")
                    dsts = [pp[:, 0, 0:512], pp[:, 1, 0:256]]
                for half, (ncol, nlen) in enumerate(((0, 512), (512, 256))):
                    nc.tensor.matmul(dsts[half], ot01[:, qs],
                                     w2a_sb[:, ncol : ncol + nlen],
                                     start=True, stop=False)
                    nc.tensor.matmul(dsts[half], ot2[:, qs],
                                     w2b_sb[:, ncol : ncol + nlen],
                                     start=False, stop=True)
                ostage = stage.tile([128, D], BF16, tag="ostage")
                nc.vector.tensor_copy(ostage[:, 0:512], dsts[0])
                if act_copy:
                    nc.scalar.copy(ostage[:, 512:768], dsts[1])
                else:
                    nc.vector.tensor_copy(ostage[:, 512:768], dsts[1])
                nc.sync.dma_start(out=out[qs, :], in_=ostage)

            op_queue = []

            def pop_op():
                if op_queue:
                    emit_one_op(op_queue.pop(0))

            # ---- attention ----
            def emit_pair_stream(sup, pairs, kt, qt, vn_h, shared_otp, drip_at):
                qbase = 512 * sup
                otp0 = ps_ot.tile([128, 512], F32, tag="ot", name="otp0")
                otp1 = otp0 if shared_otp else ps_ot.tile(
                    [128, 512], F32, tag="ot", name="otp1")
                otps = [otp0, otp1]
                for pi, (jA, jB) in enumerate(pairs):
                    c0A = max(0, 128 * (jA - 4 * sup))
                    c0B = max(0, 128 * (jB - 4 * sup))
                    sc = ps_sc.tile([128, 2, 512], F32, tag="sc")
                    nc.tensor.matmul(
                        sc[:, 0, c0A:512], kt[0:64, bass.ts(jA, 128)],
                        qt[0:64, qbase + c0A : qbase + 512],
                        start=True, stop=True, tile_position=(0, 0),
                    )
                    nc.tensor.matmul(
                        sc[:, 1, c0B:512], kt[64:128, bass.ts(jB, 128)],
                        qt[64:128, qbase + c0B : qbase + 512],
                        start=True, stop=True, tile_position=(64, 0),
                    )
                    ptile = ptpool.tile([128, 2, 512], BF16, tag="pt")
                    nc.scalar.activation(
                        out=ptile[:, :, c0A:512], in_=sc[:, :, c0A:512],
                        func=mybir.ActivationFunctionType.Exp, scale=SCALE,
                    )
                    if jB >= 4 * sup:  # diagonal: zero triangles (+ B stale)
                        if jA == jB - 1:
                            nc.gpsimd.affine_select(
                                out=ptile[:, :, c0A : c0A + 256],
                                in_=ptile[:, :, c0A : c0A + 256],
                                pattern=[[-128, 2], [1, 256]],
                                compare_op=mybir.AluOpType.is_ge,
                                fill=0.0, base=0, channel_multiplier=-1,
                            )
                        else:
                            nc.gpsimd.affine_select(
                                out=ptile[:, :, c0A : c0A + 128],
                                in_=ptile[:, :, c0A : c0A + 128],
                                pattern=[[0, 2], [1, 128]],
                                compare_op=mybir.AluOpType.is_ge,
                                fill=0.0, base=0, channel_multiplier=-1,
                            )
                    for m, (j, c0) in enumerate(((jA, c0A), (jB, c0B))):
                        if shared_otp:
                            start = pi == 0 and m == 0
                            stop = pi == len(pairs) - 1 and m == 1
                        else:
                            start = pi == 0
                            stop = pi == len(pairs) - 1
                        nc.tensor.matmul(
                            otps[m][0:65, c0:512], vn[:, j, vn_h[m], :],
                            ptile[:, m, c0:512],
                            start=start, stop=stop, skip_group_check=True,
                        )
                    if pi in drip_at:
                        pop_op()
                return otps

            def emit_norm_a(otp):
                # stage O and l to SBUF: the otp bank frees after two quick
                # copies; recip input stays partition-0 aligned (the custom
                # DVE op mishandles shifted/PSUM-shifted operands)
                lt = norm.tile([1, 512], F32, tag="lt")
                nc.vector.tensor_copy(lt, otp[64:65, :])
                ost = norm.tile([64, 512], F32, tag="ost")
                nc.vector.tensor_copy(ost, otp[0:64, :])
                rlt = norm.tile([1, 512], F32, tag="rlt")
                nc.vector.reciprocal_approx_fast(out=rlt, in_=lt)
                rbc = norm.tile([64, 512], F32, tag="rbc")
                nc.gpsimd.partition_broadcast(rbc, rlt)
                return ost, rbc

            def emit_norm_b(ab, dst):
                # deferred so the broadcast-dependent multiply never blocks
                # ring-critical evacuations at the DVE FIFO head
                nc.vector.tensor_mul(dst, ab[0], ab[1])

            emit_qkv0()
            for sup in range(NSUP):
                nch = 4 * sup + 4
                drip01 = {3} if sup == 3 else {1, nch // 2}
                otp0, otp1 = emit_pair_stream(
                    sup, [(j, j) for j in range(nch)], k01, q01, (0, 1),
                    shared_otp=False, drip_at=drip01,
                )
                ab0 = emit_norm_a(otp0)
                ab1 = emit_norm_a(otp1)
                drip2 = {0, 1, 2} if sup == 3 else {0, 2}
                otp2, _ = emit_pair_stream(
                    sup, [(j, j + 1) for j in range(0, nch, 2)], k2d, q2d,
                    (2, 2), shared_otp=True, drip_at=drip2,
                )
                ab2 = emit_norm_a(otp2)
                if sup < NSUP - 1:
                    emit_qkv(sup + 1)
                emit_norm_b(ab0, ot01[0:64, bass.ts(sup, 512)])
                emit_norm_b(ab1, ot01[64:128, bass.ts(sup, 512)])
                emit_norm_b(ab2, ot2[0:64, bass.ts(sup, 512)])
                op_queue.extend(range(4 * sup, 4 * sup + 4))
            # tail: last 4 blocks, alternating PSUM homes, ACT helps evacuate
            while op_queue:
                i = len(op_queue)
                emit_one_op(op_queue.pop(0), alt=(i % 2 == 0), act_copy=True)

    nc.compile()
    return nc


def _get_nc():
    global _CACHED_NC
    if _CACHED_NC is None:
        _CACHED_NC = build_bass()
    return _CACHED_NC


def make_in_maps(x, Wq, Wk, Wv, Wo):
    x = np.asarray(x, dtype=np.float32)
    Wq = np.asarray(Wq, dtype=np.float32)
    Wk = np.asarray(Wk, dtype=np.float32)
    Wv = np.asarray(Wv, dtype=np.float32)
    Wo = np.asarray(Wo, dtype=np.float32)
    bf = ml_dtypes.bfloat16
    in_maps = []
    for c in range(N_CORES):
        b = c // 4
        hs = [H_PER_CORE * (c % 4) + i for i in range(H_PER_CORE)]
        xT_host = np.ascontiguousarray(x[b].T).astype(bf)  # [768, 2048]
        xTb = np.ascontiguousarray(xT_host.reshape(NKCH, 128, S))
        wqk_full = np.concatenate(
            [Wq[hs[0]], Wq[hs[1]], Wk[hs[0]], Wk[hs[1]], Wq[hs[2]], Wk[hs[2]]],
            axis=1,
        ).astype(bf)  # [768, 384]
        wqkb = np.ascontiguousarray(wqk_full.reshape(NKCH, 128, 384))
        wv_full = np.concatenate([Wv[h] for h in hs], axis=1).astype(bf)
        wvb = np.ascontiguousarray(wv_full.reshape(NKCH, 128, 192))
        w2 = np.ascontiguousarray(np.concatenate(
            [Wo[:, DK * h : DK * h + DK].T for h in hs], axis=0
        ).astype(bf))  # [192, 768]
        in_maps.append({"xT": xTb, "wqk": wqkb, "wv": wvb, "w2": w2})
    return in_maps


def run_cores(in_maps, **kwargs):
    nc = _get_nc()
    return run_bass_kernel_spmd(nc, in_maps, core_ids=list(range(N_CORES)), **kwargs)


def kernel(x, Wq, Wk, Wv, Wo, bo):
    in_maps = make_in_maps(x, Wq, Wk, Wv, Wo)
    res = run_cores(in_maps)
    bo = np.asarray(bo, dtype=np.float32)
    out = np.empty((B, S, D), dtype=np.float32)
    for b in range(B):
        acc = res.results[4 * b]["out"].astype(np.float32)
        for c in range(4 * b + 1, 4 * b + 4):
            acc = acc + res.results[c]["out"].astype(np.float32)
        out[b] = acc + bo[None, :]
    return out

